# revision 3
# baseline (speedup 1.0000x reference)
"""Trainium2 Bass kernel for nn_DirectedEdgeMessage (GNN message passing).

Computation per molecule b (B=256, A=64 atoms, E=128 edges, K=6 neighbors,
H=256 features):
  w[e]   = 1 / ||xyz[p0[e]] - xyz[p1[e]]||^2      (0 where distance == 0)
  msg[e] = sum_k w[nb[e,k]] * R[nb[e,k], :]

Strategy (data-parallel over B across 8 NeuronCores, 32 molecules/core):
  * E == 128 == PE array width, so the neighbor gather+sum is a matmul
    msg = S @ R with a per-molecule scatter matrix
    S[e,e'] = w[e'] * |{k : nb[e,k] == e'}|.
  * One-hot rows U_k[e,e'] = (nb[e,k] == e') are built on the Vector engine
    with tensor_scalar(is_equal) against a constant iota row (bf16, exact).
  * The PE transposes and K-reduces them in one shot: six accumulating
    matmuls U_k.T @ I into one PSUM tile = S^T counts (fp32, exact).
  * ScalarE copies PSUM->SBUF fused with the per-partition scale w[e']
    (activation Copy with a [128,1] scale AP).
  * Main matmul runs in float32r (full-rate fp32 mode for N>=256).
  * The xyz pair gather is also a matmul: a signed one-hot lhsT
    [(side,atom)=128, e=128] against rhs [xyz; -xyz] gives diff[e, 0:3]
    directly; DVE finishes d2 -> 1/d2 with a zero-distance mask.
"""

import numpy as np
import ml_dtypes
from contextlib import ExitStack

import concourse.bass as bass
import concourse.tile as tile
from concourse import bacc, mybir
from concourse.bass_utils import run_bass_kernel_spmd

B, A, E, K, H = 256, 64, 128, 6, 256
NCORES = 8
BLOC = B // NCORES   # 32 molecules per core
GRP = 8              # molecules per DMA group (1 MiB R tile)
NGRP = BLOC // GRP

F32 = mybir.dt.float32
F32R = mybir.dt.float32r
BF16 = mybir.dt.bfloat16
I32 = mybir.dt.int32
EQ = mybir.AluOpType.is_equal
GT = mybir.AluOpType.is_gt


def _emit_pipeline(nc, tc, d, sb, pools):
    """Emit one full pass over the core's 32 molecules."""
    iota_sb, id_sb, nb_bf, pr_bf, xyz2, w_sb = (
        sb["iota"], sb["ident"], sb["nb_bf"], sb["pr_bf"], sb["xyz2"], sb["w"])
    r_t = d["r"].ap().transpose([1, 0, 2])    # [E, BLOC, H] view
    o_t = d["out"].ap().transpose([1, 0, 2])

    for g in range(NGRP):
        gb = g * GRP
        r_sb = pools["r"].tile([E, GRP * H], F32R, tag="r")
        nc.sync.dma_start(r_sb[:], r_t[:, gb:gb + GRP, :])

        # ---- Phase A: distance weights for the group's 8 molecules ----
        ps_d = pools["psd"].tile([E, GRP * 3], F32, tag="psd")
        for half in range(2):
            ps_p = pools["psp"].tile([E, 4 * E], F32, tag="psp")
            for q in range(4):
                bb = half * 4 + q          # molecule index within group
                b = gb + bb
                poh = pools["poh"].tile([E, E], BF16, tag="poh")
                nc.vector.tensor_scalar(
                    poh[:, 0:A], iota_sb[:, 0:A], pr_bf[:, 2 * b:2 * b + 1],
                    None, op0=EQ)
                nc.vector.tensor_scalar(
                    poh[:, A:2 * A], iota_sb[:, 0:A], pr_bf[:, 2 * b + 1:2 * b + 2],
                    None, op0=EQ)
                nc.tensor.matmul(ps_p[:, q * E:(q + 1) * E], poh[:], id_sb[:],
                                 start=True, stop=True)
            pt_sb = pools["pt"].tile([E, 4 * E], F32, tag="pt")
            nc.scalar.copy(pt_sb[:], ps_p[:])
            for q in range(4):
                bb = half * 4 + q
                b = gb + bb
                nc.tensor.matmul(ps_d[:, bb * 3:(bb + 1) * 3],
                                 pt_sb[:, q * E:(q + 1) * E],
                                 xyz2[:, b * 3:(b + 1) * 3],
                                 start=True, stop=True)
        sq = pools["sq"].tile([E, GRP * 3], F32, tag="sq")
        nc.scalar.square(sq[:], ps_d[:])
        d2a = pools["sq"].tile([E, GRP], F32, tag="d2a")
        nc.vector.tensor_add(d2a[:], sq[:, 0:GRP * 3:3], sq[:, 1:GRP * 3:3])
        d2 = pools["sq"].tile([E, GRP], F32, tag="d2")
        nc.vector.tensor_add(d2[:], d2a[:], sq[:, 2:GRP * 3:3])
        d2c = pools["sq"].tile([E, GRP], F32, tag="d2c")
        nc.vector.tensor_scalar_max(d2c[:], d2[:], 1e-20)
        winv = pools["sq"].tile([E, GRP], F32, tag="winv")
        nc.vector.reciprocal_approx_fast(winv[:], d2c[:])
        msk = pools["sq"].tile([E, GRP], F32, tag="msk")
        nc.vector.tensor_scalar(msk[:], d2[:], 0.0, None, op0=GT)
        nc.vector.tensor_mul(w_sb[:, gb:gb + GRP], winv[:], msk[:])

        # ---- Phase B: scatter matrices + message matmuls ----
        msg_sb = pools["msg"].tile([E, GRP * H], F32, tag="msg")
        for p2 in range(GRP // 2):
            ps_mm = pools["psmm"].tile([E, 2 * H], F32, tag="psmm")
            for o in range(2):
                bb = p2 * 2 + o
                b = gb + bb
                u = pools["u"].tile([E, K * E], BF16, tag="u")
                for k in range(K):
                    nc.vector.tensor_scalar(
                        u[:, k * E:(k + 1) * E], iota_sb[:],
                        nb_bf[:, b * K + k:b * K + k + 1], None, op0=EQ)
                ps_st = pools["psst"].tile([E, E], F32, tag="psst")
                for k in range(K):
                    nc.tensor.matmul(ps_st[:], u[:, k * E:(k + 1) * E], id_sb[:],
                                     start=(k == 0), stop=(k == K - 1))
                stw = pools["stw"].tile([E, E], F32R, tag="stw")
                nc.scalar.mul(stw[:], ps_st[:], w_sb[:, b:b + 1])
                nc.tensor.matmul(ps_mm[:, o * H:(o + 1) * H],
                                 stw[:], r_sb[:, bb * H:(bb + 1) * H],
                                 start=True, stop=True)
            nc.scalar.copy(msg_sb[:, p2 * 2 * H:(p2 + 1) * 2 * H], ps_mm[:])
        nc.sync.dma_start(o_t[:, gb:gb + GRP, :], msg_sb[:])


def build_program(loop_iters=None, body_unroll=8):
    """Build the per-core Bass program. loop_iters=None emits one straight-line
    pass (production). loop_iters=N wraps body_unroll passes in a For_i(0,N)
    device loop — used only for wall-clock timing via iteration deltas."""
    nc = bacc.Bacc("TRN2", target_bir_lowering=False, debug=False)

    d = {
        "r": nc.dram_tensor("r", [BLOC, E, H], F32R, kind="ExternalInput"),
        "nbt": nc.dram_tensor("nbt", [E, BLOC, K], I32, kind="ExternalInput"),
        "prt": nc.dram_tensor("prt", [E, BLOC, 2], I32, kind="ExternalInput"),
        "xyzt": nc.dram_tensor("xyzt", [A, BLOC, 3], F32, kind="ExternalInput"),
        "out": nc.dram_tensor("out", [BLOC, E, H], F32, kind="ExternalOutput"),
    }
    iota_np = np.broadcast_to(np.arange(E, dtype=np.float32), (E, E))
    c_iota = nc.inline_tensor(
        np.ascontiguousarray(iota_np.astype(ml_dtypes.bfloat16)), "c_iota")
    c_id = nc.inline_tensor(
        np.eye(E, dtype=np.float32).astype(ml_dtypes.bfloat16), "c_ident")

    with tile.TileContext(nc) as tc, ExitStack() as ctx:
        cpool = ctx.enter_context(tc.tile_pool(name="const", bufs=1))
        iota_sb = cpool.tile([E, E], BF16, tag="iota")
        nc.sync.dma_start(iota_sb[:], c_iota.ap()[:])
        id_sb = cpool.tile([E, E], BF16, tag="ident")
        nc.sync.dma_start(id_sb[:], c_id.ap()[:])
        nb_i = cpool.tile([E, BLOC * K], I32, tag="nbi")
        nc.sync.dma_start(nb_i[:], d["nbt"].ap()[:])
        pr_i = cpool.tile([E, BLOC * 2], I32, tag="pri")
        nc.sync.dma_start(pr_i[:], d["prt"].ap()[:])
        xyz2 = cpool.tile([E, BLOC * 3], F32, tag="xyz2")
        nc.sync.dma_start(xyz2[0:A, :], d["xyzt"].ap()[:])
        nc.sync.dma_start(xyz2[A:2 * A, :], d["xyzt"].ap()[:])

        nb_bf = cpool.tile([E, BLOC * K], F32, tag="nbbf")
        nc.vector.tensor_copy(nb_bf[:], nb_i[:])
        pr_bf = cpool.tile([E, BLOC * 2], F32, tag="prbf")
        nc.vector.tensor_copy(pr_bf[:], pr_i[:])
        nc.vector.tensor_scalar_mul(xyz2[A:2 * A, :], xyz2[A:2 * A, :], -1.0)
        w_sb = cpool.tile([E, BLOC], F32, tag="w")

        sb = {"iota": iota_sb, "ident": id_sb, "nb_bf": nb_bf, "pr_bf": pr_bf,
              "xyz2": xyz2, "w": w_sb}
        pools = {
            "r": ctx.enter_context(tc.tile_pool(name="r", bufs=2)),
            "msg": ctx.enter_context(tc.tile_pool(name="msg", bufs=2)),
            "poh": ctx.enter_context(tc.tile_pool(name="poh", bufs=2)),
            "pt": ctx.enter_context(tc.tile_pool(name="pt", bufs=2)),
            "u": ctx.enter_context(tc.tile_pool(name="u", bufs=3)),
            "stw": ctx.enter_context(tc.tile_pool(name="stw", bufs=3)),
            "sq": ctx.enter_context(tc.tile_pool(name="sq", bufs=2)),
            "psp": ctx.enter_context(tc.tile_pool(name="psp", bufs=2, space="PSUM")),
            "psd": ctx.enter_context(tc.tile_pool(name="psd", bufs=1, space="PSUM")),
            "psst": ctx.enter_context(tc.tile_pool(name="psst", bufs=2, space="PSUM")),
            "psmm": ctx.enter_context(tc.tile_pool(name="psmm", bufs=2, space="PSUM")),
        }
        if loop_iters is None:
            _emit_pipeline(nc, tc, d, sb, pools)
        else:
            with tc.For_i(0, loop_iters, 1):
                for _ in range(body_unroll):
                    _emit_pipeline(nc, tc, d, sb, pools)

    nc.compile()
    return nc


def _round_fp32r(x):
    """Round fp32 to the fp32r operand encoding (mantissa rounded to 12 bits,
    round-to-nearest; matches walrus fp32_to_fp32r). This is the operand cast
    for the PE's full-rate fp32r matmul mode — same values an on-device cast
    would produce."""
    u = x.view(np.uint32)
    add = np.uint32(0x7FF) + ((u >> np.uint32(12)) & np.uint32(1))
    return ((u + add) & np.uint32(0xFFFFF000)).view(np.float32)


def shard_inputs(bond_representations, bond_pairs, bond_neighbors, xyz):
    in_maps = []
    for c in range(NCORES):
        sl = slice(c * BLOC, (c + 1) * BLOC)
        in_maps.append({
            "r": _round_fp32r(
                np.ascontiguousarray(bond_representations[0, sl], dtype=np.float32)),
            "nbt": np.ascontiguousarray(
                np.transpose(bond_neighbors[sl], (1, 0, 2)), dtype=np.int32),
            "prt": np.ascontiguousarray(
                np.transpose(bond_pairs[sl], (1, 0, 2)), dtype=np.int32),
            "xyzt": np.ascontiguousarray(
                np.transpose(xyz[sl], (1, 0, 2)), dtype=np.float32),
        })
    return in_maps


_PROG_CACHE = {}


def _get_program(key=(None, 8)):
    if key not in _PROG_CACHE:
        _PROG_CACHE[key] = build_program(loop_iters=key[0], body_unroll=key[1])
    return _PROG_CACHE[key]


def kernel(**inputs):
    args = {k: np.asarray(v) for k, v in inputs.items()}
    in_maps = shard_inputs(args["bond_representations"], args["bond_pairs"],
                           args["bond_neighbors"], args["xyz"])
    nc = _get_program()
    res = run_bass_kernel_spmd(nc, in_maps, list(range(NCORES)))
    out = np.concatenate([res.results[c]["out"] for c in range(NCORES)], axis=0)
    return out[None].astype(np.float32)


# revision 7
# speedup vs baseline: 2.4035x; 2.4035x over previous
"""Trainium2 Bass kernel for nn_DirectedEdgeMessage (GNN message passing).

Computation per molecule b (B=256, A=64 atoms, E=128 edges, K=6 neighbors,
H=256 features):
  w[e]   = 1 / ||xyz[p0[e]] - xyz[p1[e]]||^2      (0 where distance == 0)
  msg[e] = sum_k w[nb[e,k]] * R[nb[e,k], :]

Strategy (data-parallel over B across 8 NeuronCores, 32 molecules/core):
  * E == 128 == PE array width, so the neighbor gather+sum is a matmul
    msg = S @ R with a per-molecule scatter matrix
    S[e,e'] = w[e'] * |{k : nb[e,k] == e'}|.
  * One-hot rows U_k[e,e'] = (nb[e,k] == e') are built on the Vector engine
    with tensor_scalar(is_equal) against a constant iota row (bf16, exact).
  * The PE transposes and K-reduces them in one shot: six accumulating
    matmuls U_k.T @ I into one PSUM tile = S^T counts (fp32, exact).
  * ScalarE copies PSUM->SBUF fused with the per-partition scale w[e']
    (activation Copy with a [128,1] scale AP).
  * Main matmul runs in float32r (full-rate fp32 mode for N>=256).
  * The xyz pair gather is also a matmul: a signed one-hot lhsT
    [(side,atom)=128, e=128] against rhs [xyz; -xyz] gives diff[e, 0:3]
    directly; DVE finishes d2 -> 1/d2 with a zero-distance mask.
"""

import numpy as np
import ml_dtypes
from contextlib import ExitStack

import concourse.bass as bass
import concourse.tile as tile
from concourse import bacc, mybir
from concourse.bass_utils import run_bass_kernel_spmd

B, A, E, K, H = 256, 64, 128, 6, 256
NCORES = 8
BLOC = B // NCORES   # 32 molecules per core
GRP = 8              # molecules per DMA group (1 MiB R tile)
NGRP = BLOC // GRP

F32 = mybir.dt.float32
F32R = mybir.dt.float32r
BF16 = mybir.dt.bfloat16
I32 = mybir.dt.int32
EQ = mybir.AluOpType.is_equal
GT = mybir.AluOpType.is_gt


def _emit_pipeline(nc, tc, d, sb, pools):
    """Emit one full pass over the core's 32 molecules.

    Phase A (all groups first): distance-weight chains. Phase B: scatter
    matrices + message matmuls. Emitting all of A before B maximizes the
    Tile scheduler's lookahead so A(g+1) overlaps B(g)."""
    iota_sb, id_sb, nb_bf, pr_bf, xyz2, w_sb = (
        sb["iota"], sb["ident"], sb["nb_bf"], sb["pr_bf"], sb["xyz2"], sb["w"])
    r_t = d["r"].ap().transpose([1, 0, 2])    # [E, BLOC, H] view
    o_t = d["out"].ap().transpose([1, 0, 2])

    for g in range(NGRP):
        gb = g * GRP
        # ---- Phase A: distance weights for the group's 8 molecules ----
        ps_d = pools["psd"].tile([E, GRP * 3], F32, tag="psd")
        for half in range(2):
            ps_p = pools["psp"].tile([E, 4 * E], F32, tag="psp")
            for q in range(4):
                bb = half * 4 + q          # molecule index within group
                b = gb + bb
                poh = pools["poh"].tile([E, E], BF16, tag="poh")
                eng_p = nc.vector if (b % 2 == 0) else nc.gpsimd
                eng_p.tensor_scalar(
                    poh[:, 0:A], iota_sb[:, 0:A], pr_bf[:, 2 * b:2 * b + 1],
                    None, op0=EQ)
                eng_p.tensor_scalar(
                    poh[:, A:2 * A], iota_sb[:, 0:A], pr_bf[:, 2 * b + 1:2 * b + 2],
                    None, op0=EQ)
                nc.tensor.matmul(ps_p[:, q * E:(q + 1) * E], poh[:], id_sb[:],
                                 start=True, stop=True)
            pt_sb = pools["pt"].tile([E, 4 * E], F32, tag="pt")
            nc.scalar.copy(pt_sb[:], ps_p[:])
            for q in range(4):
                bb = half * 4 + q
                b = gb + bb
                nc.tensor.matmul(ps_d[:, bb * 3:(bb + 1) * 3],
                                 pt_sb[:, q * E:(q + 1) * E],
                                 xyz2[:, b * 3:(b + 1) * 3],
                                 start=True, stop=True)
        sq = pools["sq"].tile([E, GRP * 3], F32, tag="sq")
        nc.scalar.square(sq[:], ps_d[:])
        d2a = pools["sq"].tile([E, GRP], F32, tag="d2a")
        nc.vector.tensor_add(d2a[:], sq[:, 0:GRP * 3:3], sq[:, 1:GRP * 3:3])
        d2 = pools["sq"].tile([E, GRP], F32, tag="d2")
        nc.vector.tensor_add(d2[:], d2a[:], sq[:, 2:GRP * 3:3])
        d2c = pools["sq"].tile([E, GRP], F32, tag="d2c")
        nc.vector.tensor_scalar_max(d2c[:], d2[:], 1e-20)
        winv = pools["sq"].tile([E, GRP], F32, tag="winv")
        nc.vector.reciprocal_approx_fast(winv[:], d2c[:])
        msk = pools["sq"].tile([E, GRP], F32, tag="msk")
        nc.vector.tensor_scalar(msk[:], d2[:], 0.0, None, op0=GT)
        nc.vector.tensor_mul(w_sb[:, gb:gb + GRP], winv[:], msk[:])

    for g in range(NGRP):
        gb = g * GRP
        # ---- Phase B: scatter matrices + message matmuls ----
        r_sb = pools["r"].tile([E, GRP * H], F32R, tag="r")
        nc.sync.dma_start(r_sb[:], r_t[:, gb:gb + GRP, :])
        msg_sb = pools["msg"].tile([E, GRP * H], F32, tag="msg")
        for p2 in range(GRP // 2):
            ps_mm = pools["psmm"].tile([E, 2 * H], F32, tag="psmm")
            for o in range(2):
                bb = p2 * 2 + o
                b = gb + bb
                u = pools["u"].tile([E, K * E], BF16, tag="u")
                for k in range(K):
                    eng_u = nc.gpsimd if k == K - 1 else nc.vector
                    eng_u.tensor_scalar(
                        u[:, k * E:(k + 1) * E], iota_sb[:],
                        nb_bf[:, b * K + k:b * K + k + 1], None, op0=EQ)
                ps_st = pools["psst"].tile([E, E], F32, tag="psst")
                for k in range(K):
                    nc.tensor.matmul(ps_st[:], u[:, k * E:(k + 1) * E], id_sb[:],
                                     start=(k == 0), stop=(k == K - 1))
                stw = pools["stw"].tile([E, E], F32R, tag="stw")
                nc.scalar.mul(stw[:], ps_st[:], w_sb[:, b:b + 1])
                nc.tensor.matmul(ps_mm[:, o * H:(o + 1) * H],
                                 stw[:], r_sb[:, bb * H:(bb + 1) * H],
                                 start=True, stop=True)
            if p2 % 4 == 3:
                nc.vector.tensor_copy(msg_sb[:, p2 * 2 * H:(p2 + 1) * 2 * H], ps_mm[:])
            else:
                nc.scalar.copy(msg_sb[:, p2 * 2 * H:(p2 + 1) * 2 * H], ps_mm[:])
        nc.scalar.dma_start(o_t[:, gb:gb + GRP, :], msg_sb[:])


def build_program(loop_iters=None, body_unroll=8):
    """Build the per-core Bass program. loop_iters=None emits one straight-line
    pass (production). loop_iters=N wraps body_unroll passes in a For_i(0,N)
    device loop — used only for wall-clock timing via iteration deltas."""
    nc = bacc.Bacc("TRN2", target_bir_lowering=False, debug=False)

    d = {
        "r": nc.dram_tensor("r", [BLOC, E, H], F32R, kind="ExternalInput"),
        "nbt": nc.dram_tensor("nbt", [E, BLOC, K], I32, kind="ExternalInput"),
        "prt": nc.dram_tensor("prt", [E, BLOC, 2], I32, kind="ExternalInput"),
        "xyzt": nc.dram_tensor("xyzt", [A, BLOC, 3], F32, kind="ExternalInput"),
        "out": nc.dram_tensor("out", [BLOC, E, H], F32, kind="ExternalOutput"),
    }
    iota_np = np.broadcast_to(np.arange(E, dtype=np.float32), (E, E))
    c_iota = nc.inline_tensor(
        np.ascontiguousarray(iota_np.astype(ml_dtypes.bfloat16)), "c_iota")
    c_id = nc.inline_tensor(
        np.eye(E, dtype=np.float32).astype(ml_dtypes.bfloat16), "c_ident")

    with tile.TileContext(nc) as tc, ExitStack() as ctx:
        cpool = ctx.enter_context(tc.tile_pool(name="const", bufs=1))
        iota_sb = cpool.tile([E, E], BF16, tag="iota")
        nc.sync.dma_start(iota_sb[:], c_iota.ap()[:])
        id_sb = cpool.tile([E, E], BF16, tag="ident")
        nc.sync.dma_start(id_sb[:], c_id.ap()[:])
        nb_i = cpool.tile([E, BLOC * K], I32, tag="nbi")
        nc.sync.dma_start(nb_i[:], d["nbt"].ap()[:])
        pr_i = cpool.tile([E, BLOC * 2], I32, tag="pri")
        nc.sync.dma_start(pr_i[:], d["prt"].ap()[:])
        xyz2 = cpool.tile([E, BLOC * 3], F32, tag="xyz2")
        nc.sync.dma_start(xyz2[0:A, :], d["xyzt"].ap()[:])
        nc.sync.dma_start(xyz2[A:2 * A, :], d["xyzt"].ap()[:])

        nb_bf = cpool.tile([E, BLOC * K], F32, tag="nbbf")
        nc.vector.tensor_copy(nb_bf[:], nb_i[:])
        pr_bf = cpool.tile([E, BLOC * 2], F32, tag="prbf")
        nc.vector.tensor_copy(pr_bf[:], pr_i[:])
        nc.vector.tensor_scalar_mul(xyz2[A:2 * A, :], xyz2[A:2 * A, :], -1.0)
        w_sb = cpool.tile([E, BLOC], F32, tag="w")

        sb = {"iota": iota_sb, "ident": id_sb, "nb_bf": nb_bf, "pr_bf": pr_bf,
              "xyz2": xyz2, "w": w_sb}
        pools = {
            "r": ctx.enter_context(tc.tile_pool(name="r", bufs=4)),
            "msg": ctx.enter_context(tc.tile_pool(name="msg", bufs=2)),
            "poh": ctx.enter_context(tc.tile_pool(name="poh", bufs=2)),
            "pt": ctx.enter_context(tc.tile_pool(name="pt", bufs=2)),
            "u": ctx.enter_context(tc.tile_pool(name="u", bufs=3)),
            "stw": ctx.enter_context(tc.tile_pool(name="stw", bufs=3)),
            "sq": ctx.enter_context(tc.tile_pool(name="sq", bufs=2)),
            "psp": ctx.enter_context(tc.tile_pool(name="psp", bufs=2, space="PSUM")),
            "psd": ctx.enter_context(tc.tile_pool(name="psd", bufs=1, space="PSUM")),
            "psst": ctx.enter_context(tc.tile_pool(name="psst", bufs=3, space="PSUM")),
            "psmm": ctx.enter_context(tc.tile_pool(name="psmm", bufs=2, space="PSUM")),
        }
        if loop_iters is None:
            _emit_pipeline(nc, tc, d, sb, pools)
        else:
            with tc.For_i(0, loop_iters, 1):
                for _ in range(body_unroll):
                    _emit_pipeline(nc, tc, d, sb, pools)

    nc.compile()
    return nc


def _round_fp32r(x):
    """Round fp32 to the fp32r operand encoding (mantissa rounded to 12 bits,
    round-to-nearest; matches walrus fp32_to_fp32r). This is the operand cast
    for the PE's full-rate fp32r matmul mode — same values an on-device cast
    would produce."""
    u = x.view(np.uint32)
    add = np.uint32(0x7FF) + ((u >> np.uint32(12)) & np.uint32(1))
    return ((u + add) & np.uint32(0xFFFFF000)).view(np.float32)


def shard_inputs(bond_representations, bond_pairs, bond_neighbors, xyz):
    in_maps = []
    for c in range(NCORES):
        sl = slice(c * BLOC, (c + 1) * BLOC)
        in_maps.append({
            "r": _round_fp32r(
                np.ascontiguousarray(bond_representations[0, sl], dtype=np.float32)),
            "nbt": np.ascontiguousarray(
                np.transpose(bond_neighbors[sl], (1, 0, 2)), dtype=np.int32),
            "prt": np.ascontiguousarray(
                np.transpose(bond_pairs[sl], (1, 0, 2)), dtype=np.int32),
            "xyzt": np.ascontiguousarray(
                np.transpose(xyz[sl], (1, 0, 2)), dtype=np.float32),
        })
    return in_maps


_PROG_CACHE = {}


def _get_program(key=(None, 8)):
    if key not in _PROG_CACHE:
        _PROG_CACHE[key] = build_program(loop_iters=key[0], body_unroll=key[1])
    return _PROG_CACHE[key]


def kernel(**inputs):
    args = {k: np.asarray(v) for k, v in inputs.items()}
    in_maps = shard_inputs(args["bond_representations"], args["bond_pairs"],
                           args["bond_neighbors"], args["xyz"])
    nc = _get_program()
    res = run_bass_kernel_spmd(nc, in_maps, list(range(NCORES)))
    out = np.concatenate([res.results[c]["out"] for c in range(NCORES)], axis=0)
    return out[None].astype(np.float32)


# revision 11
# speedup vs baseline: 3.9299x; 1.6351x over previous
"""Trainium2 Bass kernel for nn_DirectedEdgeMessage (GNN message passing).

Computation per molecule b (B=256, A=64 atoms, E=128 edges, K=6 neighbors,
H=256 features):
  w[e]   = 1 / ||xyz[p0[e]] - xyz[p1[e]]||^2      (0 where distance == 0)
  msg[e] = sum_k w[nb[e,k]] * R[nb[e,k], :]

Strategy (data-parallel over B across 8 NeuronCores, 32 molecules/core):
  * E == 128 == PE array width, so the neighbor gather+sum is a matmul
    msg = S @ R with a per-molecule scatter matrix
    S[e,e'] = w[e'] * |{k : nb[e,k] == e'}|.
  * One-hot rows U_k[e,e'] = (nb[e,k] == e') are built on the Vector engine
    with tensor_scalar(is_equal) against a constant iota row (bf16, exact).
  * The PE transposes and K-reduces them in one shot: six accumulating
    matmuls U_k.T @ I into one PSUM tile = S^T counts (fp32, exact).
  * ScalarE copies PSUM->SBUF fused with the per-partition scale w[e']
    (activation Copy with a [128,1] scale AP).
  * Main matmul runs in float32r (full-rate fp32 mode for N>=256).
  * The xyz pair gather is also a matmul: a signed one-hot lhsT
    [(side,atom)=128, e=128] against rhs [xyz; -xyz] gives diff[e, 0:3]
    directly; DVE finishes d2 -> 1/d2 with a zero-distance mask.
"""

import numpy as np
import ml_dtypes
from contextlib import ExitStack

import concourse.bass as bass
import concourse.tile as tile
from concourse import bacc, mybir
from concourse.bass_utils import run_bass_kernel_spmd

B, A, E, K, H = 256, 64, 128, 6, 256
NCORES = 8
BLOC = B // NCORES   # 32 molecules per core
GRP = 8              # molecules per DMA group (1 MiB R tile)
NGRP = BLOC // GRP

F32 = mybir.dt.float32
F32R = mybir.dt.float32r
BF16 = mybir.dt.bfloat16
I32 = mybir.dt.int32
EQ = mybir.AluOpType.is_equal
GT = mybir.AluOpType.is_gt

# Experiment knobs (overridable before build_program):
#   idmm: transposing matmuls per molecule (6 = no pre-reduce, 3/2/1 = DVE
#         scalar_tensor_tensor chains fold one-hots first)
#   gps_u: route the last one-hot chain to GPSIMD
#   gps_p: route odd molecules' pair one-hots to GPSIMD
CFG = {"idmm": 3, "gps_u": False, "gps_p": False}


def _emit_pipeline(nc, tc, d, sb, pools):
    """Emit one full pass over the core's 32 molecules.

    Phase A (all groups first): distance-weight chains. Phase B: scatter
    matrices + message matmuls. Emitting all of A before B maximizes the
    Tile scheduler's lookahead so A(g+1) overlaps B(g)."""
    iota_sb, id_sb, nb_bf, pr_bf, xyz2, w_sb = (
        sb["iota"], sb["ident"], sb["nb_bf"], sb["pr_bf"], sb["xyz2"], sb["w"])
    r_t = d["r"].ap().transpose([1, 0, 2])    # [E, BLOC, H] view
    o_t = d["out"].ap().transpose([1, 0, 2])

    for g in range(NGRP):
        gb = g * GRP
        # ---- Phase A: distance weights for the group's 8 molecules ----
        ps_d = pools["psp"].tile([E, GRP * 3], F32, tag="psp")
        for half in range(2):
            ps_p = pools["psp"].tile([E, 4 * E], F32, tag="psp")
            for q in range(4):
                bb = half * 4 + q          # molecule index within group
                b = gb + bb
                poh = pools["poh"].tile([E, E], BF16, tag="poh")
                eng_p = nc.vector if (b % 2 == 0 or not CFG["gps_p"]) else nc.gpsimd
                eng_p.tensor_scalar(
                    poh[:, 0:A], iota_sb[:, 0:A], pr_bf[:, 2 * b:2 * b + 1],
                    None, op0=EQ)
                eng_p.tensor_scalar(
                    poh[:, A:2 * A], iota_sb[:, 0:A], pr_bf[:, 2 * b + 1:2 * b + 2],
                    None, op0=EQ)
                nc.tensor.matmul(ps_p[:, q * E:(q + 1) * E], poh[:], id_sb[:],
                                 start=True, stop=True)
            pt_sb = pools["pt"].tile([E, 4 * E], F32, tag="pt")
            nc.scalar.copy(pt_sb[:], ps_p[:])
            for q in range(4):
                bb = half * 4 + q
                b = gb + bb
                nc.tensor.matmul(ps_d[:, bb * 3:(bb + 1) * 3],
                                 pt_sb[:, q * E:(q + 1) * E],
                                 xyz2[:, b * 3:(b + 1) * 3],
                                 start=True, stop=True)
        sq = pools["sq"].tile([E, GRP * 3], F32, tag="sq")
        nc.scalar.square(sq[:], ps_d[:])
        d2a = pools["sq"].tile([E, GRP], F32, tag="d2a")
        nc.vector.tensor_add(d2a[:], sq[:, 0:GRP * 3:3], sq[:, 1:GRP * 3:3])
        d2 = pools["sq"].tile([E, GRP], F32, tag="d2")
        nc.vector.tensor_add(d2[:], d2a[:], sq[:, 2:GRP * 3:3])
        d2c = pools["sq"].tile([E, GRP], F32, tag="d2c")
        nc.vector.tensor_scalar_max(d2c[:], d2[:], 1e-20)
        winv = pools["sq"].tile([E, GRP], F32, tag="winv")
        nc.vector.reciprocal_approx_fast(winv[:], d2c[:])
        nc.vector.scalar_tensor_tensor(
            w_sb[:, gb:gb + GRP], d2[:], 0.0, winv[:],
            op0=GT, op1=mybir.AluOpType.mult)

    for g in range(NGRP):
        gb = g * GRP
        # ---- Phase B: scatter matrices + message matmuls ----
        r_sb = pools["r"].tile([E, GRP * H], F32R, tag="r")
        nc.sync.dma_start(r_sb[:], r_t[:, gb:gb + GRP, :])
        msg_sb = pools["msg"].tile([E, GRP * H], F32, tag="msg")
        for p4 in range(GRP // 4):
            ps_mm = pools["psmm"].tile([E, 4 * H], F32, tag="psmm")
            for o in range(4):
                bb = p4 * 4 + o
                b = gb + bb
                # Build CH pre-reduced one-hot sums (chains of K//CH on DVE
                # via fused scalar_tensor_tensor), then CH transposing
                # accumulate-matmuls on the PE.
                ch = CFG["idmm"]               # id-matmuls per molecule
                clen = K // ch                 # one-hots folded per chain
                u = pools["u"].tile([E, ch * E], BF16, tag="u")
                for c in range(ch):
                    k0 = c * clen
                    eng_u = (nc.gpsimd if (CFG["gps_u"] and c == ch - 1)
                             else nc.vector)
                    eng_u.tensor_scalar(
                        u[:, c * E:(c + 1) * E], iota_sb[:],
                        nb_bf[:, b * K + k0:b * K + k0 + 1], None, op0=EQ)
                    for k in range(k0 + 1, k0 + clen):
                        eng_u.scalar_tensor_tensor(
                            u[:, c * E:(c + 1) * E], iota_sb[:],
                            nb_bf[:, b * K + k:b * K + k + 1],
                            u[:, c * E:(c + 1) * E],
                            op0=EQ, op1=mybir.AluOpType.add)
                ps_st = pools["psst"].tile([E, E], F32, tag="psst")
                for c in range(ch):
                    nc.tensor.matmul(ps_st[:], u[:, c * E:(c + 1) * E], id_sb[:],
                                     start=(c == 0), stop=(c == ch - 1))
                stw = pools["stw"].tile([E, E], F32R, tag="stw")
                nc.scalar.mul(stw[:], ps_st[:], w_sb[:, b:b + 1])
                nc.tensor.matmul(ps_mm[:, o * H:(o + 1) * H],
                                 stw[:], r_sb[:, bb * H:(bb + 1) * H],
                                 start=True, stop=True)
            nc.scalar.copy(msg_sb[:, p4 * 4 * H:(p4 + 1) * 4 * H], ps_mm[:])
            nc.scalar.dma_start(
                o_t[:, gb + p4 * 4:gb + (p4 + 1) * 4, :],
                msg_sb[:, p4 * 4 * H:(p4 + 1) * 4 * H])


def build_program(loop_iters=None, body_unroll=8):
    """Build the per-core Bass program. loop_iters=None emits one straight-line
    pass (production). loop_iters=N wraps body_unroll passes in a For_i(0,N)
    device loop — used only for wall-clock timing via iteration deltas."""
    nc = bacc.Bacc("TRN2", target_bir_lowering=False, debug=False)

    d = {
        "r": nc.dram_tensor("r", [BLOC, E, H], F32R, kind="ExternalInput"),
        "nbt": nc.dram_tensor("nbt", [E, BLOC, K], I32, kind="ExternalInput"),
        "prt": nc.dram_tensor("prt", [E, BLOC, 2], I32, kind="ExternalInput"),
        "xyzt": nc.dram_tensor("xyzt", [A, BLOC, 3], F32, kind="ExternalInput"),
        "out": nc.dram_tensor("out", [BLOC, E, H], F32, kind="ExternalOutput"),
    }
    iota_np = np.broadcast_to(np.arange(E, dtype=np.float32), (E, E))
    c_iota = nc.inline_tensor(
        np.ascontiguousarray(iota_np.astype(ml_dtypes.bfloat16)), "c_iota")
    c_id = nc.inline_tensor(
        np.eye(E, dtype=np.float32).astype(ml_dtypes.bfloat16), "c_ident")

    with tile.TileContext(nc) as tc, ExitStack() as ctx:
        cpool = ctx.enter_context(tc.tile_pool(name="const", bufs=1))
        pr_i = cpool.tile([E, BLOC * 2], I32, tag="pri")
        nc.sync.dma_start(pr_i[:], d["prt"].ap()[:])
        iota_sb = cpool.tile([E, E], BF16, tag="iota")
        nc.scalar.dma_start(iota_sb[:], c_iota.ap()[:])
        nb_i = cpool.tile([E, BLOC * K], I32, tag="nbi")
        nc.sync.dma_start(nb_i[:], d["nbt"].ap()[:])
        id_sb = cpool.tile([E, E], BF16, tag="ident")
        nc.scalar.dma_start(id_sb[:], c_id.ap()[:])
        xyz2 = cpool.tile([E, BLOC * 3], F32, tag="xyz2")
        nc.sync.dma_start(xyz2[0:A, :], d["xyzt"].ap()[:])
        nc.scalar.dma_start(xyz2[A:2 * A, :], d["xyzt"].ap()[:])

        nb_bf = cpool.tile([E, BLOC * K], F32, tag="nbbf")
        nc.vector.tensor_copy(nb_bf[:], nb_i[:])
        pr_bf = cpool.tile([E, BLOC * 2], F32, tag="prbf")
        nc.vector.tensor_copy(pr_bf[:], pr_i[:])
        nc.vector.tensor_scalar_mul(xyz2[A:2 * A, :], xyz2[A:2 * A, :], -1.0)
        w_sb = cpool.tile([E, BLOC], F32, tag="w")

        sb = {"iota": iota_sb, "ident": id_sb, "nb_bf": nb_bf, "pr_bf": pr_bf,
              "xyz2": xyz2, "w": w_sb}
        pools = {
            "r": ctx.enter_context(tc.tile_pool(name="r", bufs=4)),
            "msg": ctx.enter_context(tc.tile_pool(name="msg", bufs=2)),
            "poh": ctx.enter_context(tc.tile_pool(name="poh", bufs=2)),
            "pt": ctx.enter_context(tc.tile_pool(name="pt", bufs=2)),
            "u": ctx.enter_context(tc.tile_pool(name="u", bufs=3)),
            "stw": ctx.enter_context(tc.tile_pool(name="stw", bufs=3)),
            "sq": ctx.enter_context(tc.tile_pool(name="sq", bufs=2)),
            "psp": ctx.enter_context(tc.tile_pool(name="psp", bufs=2, space="PSUM")),
            "psst": ctx.enter_context(tc.tile_pool(name="psst", bufs=2, space="PSUM")),
            "psmm": ctx.enter_context(tc.tile_pool(name="psmm", bufs=2, space="PSUM")),
        }
        if loop_iters is None:
            _emit_pipeline(nc, tc, d, sb, pools)
        else:
            with tc.For_i(0, loop_iters, 1):
                for _ in range(body_unroll):
                    _emit_pipeline(nc, tc, d, sb, pools)

    nc.compile()
    return nc


def _round_fp32r(x):
    """Round fp32 to the fp32r operand encoding (mantissa rounded to 12 bits,
    round-to-nearest; matches walrus fp32_to_fp32r). This is the operand cast
    for the PE's full-rate fp32r matmul mode — same values an on-device cast
    would produce."""
    u = x.view(np.uint32)
    add = np.uint32(0x7FF) + ((u >> np.uint32(12)) & np.uint32(1))
    return ((u + add) & np.uint32(0xFFFFF000)).view(np.float32)


def shard_inputs(bond_representations, bond_pairs, bond_neighbors, xyz):
    in_maps = []
    for c in range(NCORES):
        sl = slice(c * BLOC, (c + 1) * BLOC)
        in_maps.append({
            "r": _round_fp32r(
                np.ascontiguousarray(bond_representations[0, sl], dtype=np.float32)),
            "nbt": np.ascontiguousarray(
                np.transpose(bond_neighbors[sl], (1, 0, 2)), dtype=np.int32),
            "prt": np.ascontiguousarray(
                np.transpose(bond_pairs[sl], (1, 0, 2)), dtype=np.int32),
            "xyzt": np.ascontiguousarray(
                np.transpose(xyz[sl], (1, 0, 2)), dtype=np.float32),
        })
    return in_maps


_PROG_CACHE = {}


def _get_program(key=(None, 8)):
    if key not in _PROG_CACHE:
        _PROG_CACHE[key] = build_program(loop_iters=key[0], body_unroll=key[1])
    return _PROG_CACHE[key]


def kernel(**inputs):
    args = {k: np.asarray(v) for k, v in inputs.items()}
    in_maps = shard_inputs(args["bond_representations"], args["bond_pairs"],
                           args["bond_neighbors"], args["xyz"])
    nc = _get_program()
    res = run_bass_kernel_spmd(nc, in_maps, list(range(NCORES)))
    out = np.concatenate([res.results[c]["out"] for c in range(NCORES)], axis=0)
    return out[None].astype(np.float32)
